# revision 28
# baseline (speedup 1.0000x reference)
"""Self-contained Bass/Trainium2 kernel for nn_Attention (B=4, N=2048, D=1024, H=16, dh=64).

Sharding: 8 cores = (batch b in 0..3) x (head-group hg in 0..1), i.e. tensor
parallelism over heads inside each batch pair (per the to_qkv column / to_out
row sharding hint). Each core projects q/k/v for its 8 heads over the FULL
sequence (so no K/V duplication and no cross-core communication), runs
attention for those heads over all 2048 queries, and computes the row-sharded
half of the output projection. The two partial [2048, 1024] outputs of a pair
are summed on the host during unshard (bias is added on the hg=0 core; the
hg=1 core receives a zero bias so one SPMD program serves all cores).

This head split halves the K and V projection matmul work per core vs the
seq-split variant (which must build full-sequence K/V on both pair cores):
PE streaming work drops from ~918K to ~786K cycles/core.

Layout: all matmul operands fp16 (PSUM f32). V is projected directly in
keys-major layout (stationary = x^T blocks, moving = Wv) so no PE transposes
are needed. Each per-head V block carries 64 ones columns, so the AV matmul
emits the softmax row-sums replicated across PSUM partitions 64..127 for
free; the normalization is then one reciprocal_approx_fast + one multiply on
DVE. Projection work is split into PSUM-tile-sized units and interleaved into
the attention loop (V production streams inside the first attention unit at
one key-tile per step, just ahead of its consumption by the AV chain). The
output projection is split 3+1 over its 4 contraction tiles: the 3-tile half
(heads 0-5, ready after group 2) threads through the last attention group;
only the 1-tile tail trails the attention, overlapped with the output DMA.
"""

import sys
import numpy as np

sys.path.insert(0, "/opt/trn_rl_repo")

B, N, DIM = 4, 2048, 1024
HEADS, DH = 16, 64
NH = 8                # heads per core
SCALE = DH ** -0.5    # 0.125
NC = 8
HALF = N // 2

_compiled = None


def _build():
    import concourse.tile as tile
    from concourse import bacc, mybir

    f32 = mybir.dt.float32
    f16 = mybir.dt.float16
    EXP = mybir.ActivationFunctionType.Exp

    nc = bacc.Bacc("TRN2", target_bir_lowering=False, debug=False, num_devices=NC)

    CT = DIM // 128       # 8 contraction tiles over input channels
    MT = NH * DH // 128   # 4 dim tiles (head-pairs) for kT/qT/ctx
    JT = N // 128         # 16 key tiles
    VW = 128              # per-head v block: 64 dims + 64 ones columns

    # w_q/w_k arrive host-prepacked as [p, m, ct, d] so each m-tile is one
    # contiguous-line DMA (the natural [D, 512] layout would need 256B
    # strided elements — 4x the descriptors and ~4x the issue time).
    X = nc.dram_tensor("x", (DIM, N), f16, kind="ExternalInput").ap()
    WQ = nc.dram_tensor("w_q", (128, MT * CT * 128), f16,
                        kind="ExternalInput").ap()
    WK = nc.dram_tensor("w_k", (128, MT * CT * 128), f16,
                        kind="ExternalInput").ap()
    WV = nc.dram_tensor("w_v", (DIM, NH * DH), f16, kind="ExternalInput").ap()
    WO = nc.dram_tensor("w_out", (NH * DH, DIM), f16, kind="ExternalInput").ap()
    BOUT = nc.dram_tensor("b_out", (DIM,), f32, kind="ExternalInput").ap()
    Y = nc.dram_tensor("y", (N, DIM), f16, kind="ExternalOutput").ap()

    with tile.TileContext(nc) as tc:
        with tc.tile_pool(name="persist", bufs=1) as persist, \
             tc.tile_pool(name="attnbuf", bufs=1) as attnbuf, \
             tc.tile_pool(name="wpool", bufs=1) as wpool:

            kT = [persist.tile([128, N], f16, tag="kT", bufs=MT, name=f"kT{m}")
                  for m in range(MT)]
            qT = [persist.tile([128, N], f16, tag="qT", bufs=MT,
                               name=f"qT{m}") for m in range(MT)]
            v_ext = [persist.tile([128, NH * VW], f16, tag="vext", bufs=JT,
                                  name=f"vext{t}") for t in range(JT)]
            ctx = [persist.tile([128, N], f16, tag="ctx", bufs=MT,
                                name=f"ctx{m}") for m in range(MT)]

            # bias broadcast to all partitions once (zeros on hg=1 cores)
            bias_src = persist.tile([1, DIM], f32, tag="bias_src")
            nc.sync.dma_start(bias_src[:], BOUT.rearrange("(o d) -> o d", o=1))
            bias = persist.tile([128, DIM], f32, tag="bias")
            nc.gpsimd.partition_broadcast(bias[:], bias_src[0:1, :])

            # prefire the exp table load off the critical path
            dummy = attnbuf.tile([1, 8], f16, tag="dummy")
            nc.scalar.activation(dummy[:], bias_src[0:1, 0:8], EXP,
                                 bias=0.0, scale=1.0)

            # ones columns of v_ext (disjoint from the V-projection writes)
            for t in range(JT):
                ones_col = v_ext[t].rearrange(
                    "p (hh c) -> p hh c", c=VW)[:, :, DH:VW]
                nc.gpsimd.memset(ones_col, 1.0)

            with tc.tile_pool(name="psB", bufs=1, space="PSUM") as psB, \
                 tc.tile_pool(name="psInt", bufs=1, space="PSUM") as psInt:
                stage_cm = tc.tile_pool(name="stage", bufs=1)
                stage = stage_cm.__enter__()

                def w_col(src, m):
                    """[128, 8, 128] view of prepacked src for m-tile m."""
                    return src[:, m * CT * 128:(m + 1) * CT * 128].rearrange(
                        "p (t d) -> p t d", d=128)

                # ---- weight + x staging, ordered for fastest first matmul.
                # Few large contiguous-line DMAs: instruction issue (~0.7us
                # per DMA_DIRECT2D) dominates the startup, not bandwidth.
                # x tiles alternate between both HWDGE queues (SP +
                # Activation) — the scalar engine is idle until the first exp.
                wt_k = [wpool.tile([128, CT, 128], f16, tag="wkq",
                                   bufs=2 * MT, name=f"wk{m}") for m in range(MT)]
                wt_q = [wpool.tile([128, CT, 128], f16, tag="wkq",
                                   bufs=2 * MT, name=f"wq{m}") for m in range(MT)]
                xbT = [stage.tile([128, N], f16, tag="xbT", bufs=CT,
                                  name=f"xbT{ct}") for ct in range(CT)]
                wv_all = stage.tile([128, CT, 512], f16, tag="wv")

                nc.sync.dma_start(wt_k[0][:], w_col(WK, 0))
                for ct in range(CT):
                    eng = nc.scalar if ct % 2 else nc.sync
                    eng.dma_start(xbT[ct][:], X[ct * 128:(ct + 1) * 128, :])
                nc.sync.dma_start(wt_q[0][:], w_col(WQ, 0))
                nc.scalar.dma_start(
                    wv_all[:], WV.rearrange("(c p) d -> p c d", p=128))
                nc.sync.dma_start(wt_k[1][:], w_col(WK, 1))
                nc.sync.dma_start(wt_q[1][:], w_col(WQ, 1))
                for m in (2, 3):
                    nc.sync.dma_start(wt_k[m][:], w_col(WK, m))
                    nc.sync.dma_start(wt_q[m][:], w_col(WQ, m))

                # ---- projection units: one PSUM-tile lifecycle each ----
                def carrier(ps, src=None):
                    """1x1 wait-carrier matmul into ps's corner.

                    Absorbs the cross-engine semaphore waits (PSUM-buffer WAR
                    on a DVE/ACT consumer, and src's producer) that would
                    otherwise ride on the next real matmul: add_sem_waits
                    elides the now-redundant waits there, so the real
                    matmuls' LDWEIGHTS stay eligible for the PE's pull-ahead
                    window instead of stalling ~95ns at every
                    accumulation-group entry.  The written element is
                    overwritten by the group's start=True bank clear.
                    """
                    if src is None:
                        src = xbT[0][0:1, 0:1]
                    nc.tensor.matmul(ps[0:1, 0:1], src, src,
                                     start=True, stop=True)

                def kq_unit(wt, dst, s):
                    """dst[:, s*512 : +512] = W[:, m-block].T @ x^T."""
                    ps = psInt.tile([128, 512], f32, tag="pint", bufs=2,
                                    name=f"pi_{dst.name}_{s}")
                    carrier(ps)
                    for ct in range(CT):
                        nc.tensor.matmul(ps[:],
                                         wt[:, ct, :],
                                         xbT[ct][:, s * 512:(s + 1) * 512],
                                         start=(ct == 0), stop=(ct == CT - 1))
                    nc.vector.tensor_copy(
                        dst[:, s * 512:(s + 1) * 512], ps[:])

                def v_unit(t):
                    """v_ext[t] (all 8 heads) from x^T key block t."""
                    ps = psInt.tile([128, 512], f32, tag="pint", bufs=2,
                                    name=f"pv{t}")
                    carrier(ps)
                    for ct in range(CT):
                        nc.tensor.matmul(
                            ps[:],
                            xbT[ct][:, t * 128:(t + 1) * 128],
                            wv_all[:, ct, :],
                            start=(ct == 0), stop=(ct == CT - 1))
                    dst = v_ext[t].rearrange("p (hh c) -> p hh c", c=VW)[
                        :, :, 0:DH]
                    nc.vector.tensor_copy(dst, ps.rearrange(
                        "p (hh c) -> p hh c", c=DH))

                pending = []

                # ---- attention for one (head, query-half) unit ----
                def attn_unit(h, sq, interleave, u_idx, pace=4,
                              inline_v=None, last=False):
                    hp, p = divmod(h, 2)
                    po = psB.tile([128, 1024], f32, tag="po", bufs=1,
                                  name=f"po{h}_{sq}")
                    ats = {}

                    def av(j):
                        at = ats.pop(j)
                        for u in (0, 1):
                            nc.tensor.matmul(
                                po[:, u * 512:(u + 1) * 512],
                                v_ext[j][:, h * VW:(h + 1) * VW],
                                at[:, u * 512:(u + 1) * 512],
                                start=(j == 0), stop=(j == JT - 1))

                    # AV runs in lagged pairs (av(jt-3), av(jt-2) at odd jt):
                    # the second av of a pair continues the po accumulation
                    # chain back-to-back, so its LDWEIGHTS pipelines like a
                    # mid-chain load instead of paying the ~95ns group-entry
                    # stall.  Lag 3/2 keeps the exp producer well ahead.
                    for jt in range(JT):
                        pp = psB.tile([128, 1024], f32, tag="pp", bufs=2,
                                      name=f"pp{h}_{sq}_{jt}")
                        if jt >= 2:
                            carrier(pp, ats[jt - 2][0:1, 0:1])
                        for u in (0, 1):
                            nc.tensor.matmul(
                                pp[:, u * 512:(u + 1) * 512],
                                kT[hp][p * 64:(p + 1) * 64,
                                       jt * 128:(jt + 1) * 128],
                                qT[hp][p * 64:(p + 1) * 64,
                                       sq * 1024 + u * 512:
                                       sq * 1024 + (u + 1) * 512],
                                start=True, stop=True)
                        at = attnbuf.tile([128, 1024], f16, tag="at", bufs=4,
                                          name=f"at{h}_{sq}_{jt}")
                        nc.scalar.activation(at[:], pp[:], EXP,
                                             bias=0.0, scale=SCALE)
                        ats[jt] = at
                        if jt % 2 == 1 and jt >= 3:
                            av(jt - 3)
                            av(jt - 2)
                        if inline_v:
                            for u in inline_v.get(jt, ()):
                                u()
                        if interleave:
                            want = (u_idx * JT + jt + 1) * interleave[0] \
                                // (pace * JT)
                            while interleave[0] - len(pending) < want and pending:
                                pending.pop(0)()
                    av(JT - 2)
                    av(JT - 1)
                    # evacuate po with two quick copies so the next unit's AV
                    # chain gets the PSUM bank back ASAP; the reciprocal and
                    # the normalizing multiply run off the critical path.
                    # (reciprocal_approx_fast mis-reads partition-offset PSUM
                    # APs, and SBUF*SBUF tensor ops need equal input base
                    # partitions, so both operands stage at partitions 0..63.)
                    dstc = ctx[hp][p * 64:(p + 1) * 64,
                                   sq * 1024:(sq + 1) * 1024]
                    ss = attnbuf.tile([64, 1024], f32, tag="ss", bufs=1,
                                      name=f"ss{h}_{sq}")
                    rb = attnbuf.tile([64, 1024], f32, tag="rb", bufs=1,
                                      name=f"rb{h}_{sq}")
                    if last:
                        # split by column halves so the output-projection tail
                        # (which consumes ctx columns in ascending order) can
                        # start ~2us earlier.
                        for c in (0, 1):
                            cs = slice(c * 512, (c + 1) * 512)
                            nc.vector.tensor_copy(ss[:, cs], po[64:128, cs])
                            nc.vector.reciprocal_approx_fast(rb[:, cs], ss[:, cs])
                            nc.vector.tensor_mul(dstc[:, cs], po[0:64, cs],
                                                 rb[:, cs])
                        return
                    cu = attnbuf.tile([64, 1024], f16, tag="cu", bufs=1,
                                      name=f"cu{h}_{sq}")
                    nc.vector.tensor_copy(cu[:], po[0:64, :])
                    nc.vector.tensor_copy(ss[:], po[64:128, :])
                    nc.vector.reciprocal_approx_fast(rb[:], ss[:])
                    nc.vector.tensor_mul(dstc, cu[:], rb[:])

                def kq_units_for(m):
                    return ([lambda s=s, m=m: kq_unit(wt_k[m], kT[m], s)
                             for s in range(4)]
                            + [lambda s=s, m=m: kq_unit(wt_q[m], qT[m], s)
                               for s in range(4)])

                # P0: bare minimum for attention unit (h=0, sq=0) jt 0..3;
                # kT[0]'s remaining column slices and all later V key tiles
                # stream inside the unit itself, a few steps ahead of use
                for s in (0,):
                    kq_unit(wt_k[0], kT[0], s)
                kq_unit(wt_q[0], qT[0], 0)
                kq_unit(wt_q[0], qT[0], 1)
                v_unit(0)
                inline0 = {jt: [] for jt in range(JT - 1)}
                for jt in range(JT - 1):
                    inline0[jt].append(lambda t=jt + 1: v_unit(t))
                for s in (1, 2, 3):
                    inline0[s - 1].append(
                        lambda s=s: kq_unit(wt_k[0], kT[0], s))

                # groups 0..2; group g runs heads (2g, 2g+1), both query
                # halves; P(g+1) threads through A(g)'s slots.  unit order
                # (2g,0),(2g+1,0),(2g,1),(2g+1,1) so qT[g] s2/s3 (drained
                # during the first two units) are ready for the sq=1 pair.
                for g in range(3):
                    if g == 0:
                        pending.extend(
                            [lambda: kq_unit(wt_q[0], qT[0], 2),
                             lambda: kq_unit(wt_q[0], qT[0], 3)]
                            + kq_units_for(1))
                    else:
                        pending.extend(kq_units_for(g + 1))
                    units = [(2 * g, 0), (2 * g + 1, 0),
                             (2 * g, 1), (2 * g + 1, 1)]
                    for i, (h, sq) in enumerate(units):
                        if g == 0 and i == 0:
                            attn_unit(h, sq, None, 0, inline_v=inline0)
                        elif g == 0:
                            attn_unit(h, sq, [len(pending)] if pending else None,
                                      i - 1, pace=3)
                        else:
                            attn_unit(h, sq, [len(pending)] if pending else None,
                                      i, pace=4)
                    while pending:
                        pending.pop(0)()

                stage_cm.__exit__(None, None, None)
                # stage (x^T, wv) is gone.  Output projection schedule:
                #  - rows 0..1023 (sq=0 ctx, complete after unit (7,0)):
                #    full 4-ct units + output DMA thread through attention
                #    units (6,1) and (7,1), so half the output DMA streams
                #    during the last attention stretch.
                #  - rows 1024..2047: the 3-ct part (heads 0..5, ready at
                #    group-3 entry) threads through units (6,0)/(7,0) into
                #    yps; only the 1-ct ctx[3] tail trails the attention,
                #    overlapped with the remaining output DMA.
                with tc.tile_pool(name="outw", bufs=1) as outw:
                    wo = [outw.tile([128, DIM], f16, tag="wo", bufs=MT,
                                    name=f"wo{ft}") for ft in range(MT)]
                    yps = [outw.tile([128, DIM], f16, tag="yps", bufs=JT // 2,
                                     name=f"yps{qt}") for qt in range(8, JT)]
                    for ft in range(MT):
                        nc.sync.dma_start(
                            wo[ft][:], WO[ft * 128:(ft + 1) * 128, :])

                    def yp_unit(qt, s, f0, f1):
                        """y rows qt, cols s*512 += sum(ft in f0..f1) ctx.T @ wo."""
                        ps = psInt.tile([128, 512], f32, tag="pint", bufs=2,
                                        name=f"py{qt}_{s}_{f0}")
                        carrier(ps, wo[f0][0:1, 0:1])
                        for ft in range(f0, f1):
                            nc.tensor.matmul(ps[:],
                                             ctx[ft][:, qt * 128:(qt + 1) * 128],
                                             wo[ft][:, s * 512:(s + 1) * 512],
                                             start=(ft == f0), stop=(ft == f1 - 1))
                        sl = slice(s * 512, (s + 1) * 512)
                        if f0 == 0 and f1 < MT:
                            nc.vector.tensor_add(yps[qt - 8][:, sl], ps[:],
                                                 bias[:, sl])
                            return
                        ysb = outw.tile([128, 512], f16, tag="ysb", bufs=4,
                                        name=f"ysb{qt}_{s}")
                        if f0 == 0:
                            nc.vector.tensor_add(ysb[:], ps[:], bias[:, sl])
                        else:
                            nc.vector.tensor_add(ysb[:], ps[:], yps[qt - 8][:, sl])
                        nc.sync.dma_start(
                            Y[qt * 128:(qt + 1) * 128, sl], ysb[:])

                    # rows 1024..2047: 3-ct partial (heads 0..5) + bias into
                    # yps during units (6,0)/(7,0)
                    pending.extend(lambda qt=qt, s=s: yp_unit(qt, s, 0, 3)
                                   for qt in range(8, JT) for s in (0, 1))
                    ileave = [len(pending)]
                    attn_unit(6, 0, ileave, 0, pace=2)
                    attn_unit(7, 0, ileave, 1, pace=2)
                    while pending:
                        pending.pop(0)()
                    # rows 0..1023: full 4-ct units, DMA'd during (6,1)/(7,1)
                    pending.extend(lambda qt=qt, s=s: yp_unit(qt, s, 0, MT)
                                   for qt in range(8) for s in (0, 1))
                    ileave = [len(pending)]
                    attn_unit(6, 1, ileave, 0, pace=2)
                    attn_unit(7, 1, ileave, 1, pace=2, last=True)
                    while pending:
                        pending.pop(0)()
                    # tail: ctx[3] (heads 6,7) contraction for rows 1024..2047
                    for qt in range(8, JT):
                        for s in (0, 1):
                            yp_unit(qt, s, 3, 4)

    nc.compile()
    return nc


def _get_compiled():
    global _compiled
    if _compiled is None:
        _compiled = _build()
    return _compiled


def _build_in_maps(x, w_qkv, w_out, b_out):
    x = np.asarray(x, dtype=np.float32)
    w_qkv = np.asarray(w_qkv, dtype=np.float16)
    w_out = np.asarray(w_out, dtype=np.float16)
    b_out = np.asarray(b_out, dtype=np.float32)
    zeros = np.zeros_like(b_out)

    def prepack(w):
        # [D, 512] -> [p, m, ct, d] so each m-tile DMA reads contiguous
        # 2KB-per-partition lines on device
        mt, ct = NH * DH // 128, DIM // 128
        return np.ascontiguousarray(
            w.reshape(ct, 128, mt, 128).transpose(1, 2, 0, 3).reshape(
                128, mt * ct * 128))

    xbs = [np.ascontiguousarray(x[b].T.astype(np.float16)) for b in range(B)]
    in_maps = []
    for c in range(NC):
        b, hg = divmod(c, 2)
        cols = slice(hg * NH * DH, (hg + 1) * NH * DH)
        in_maps.append({
            "x": xbs[b],
            "w_q": prepack(w_qkv[:, 0 * DIM:1 * DIM][:, cols]),
            "w_k": prepack(w_qkv[:, 1 * DIM:2 * DIM][:, cols]),
            "w_v": np.ascontiguousarray(w_qkv[:, 2 * DIM:3 * DIM][:, cols]),
            "w_out": np.ascontiguousarray(w_out[cols, :]),
            "b_out": b_out if hg == 0 else zeros,
        })
    return in_maps


def kernel(x, w_qkv, w_out, b_out):
    from concourse.bass_utils import run_bass_kernel_spmd

    nc = _get_compiled()
    in_maps = _build_in_maps(x, w_qkv, w_out, b_out)
    res = run_bass_kernel_spmd(nc, in_maps, core_ids=list(range(NC)))

    out = np.empty((B, N, DIM), dtype=np.float32)
    for b in range(B):
        out[b] = (res.results[2 * b]["y"].astype(np.float32)
                  + res.results[2 * b + 1]["y"].astype(np.float32))
    return out


# revision 34
# speedup vs baseline: 1.0771x; 1.0771x over previous
"""Self-contained Bass/Trainium2 kernel for nn_Attention (B=4, N=2048, D=1024, H=16, dh=64).

Sharding: 8 cores = (batch b in 0..3) x (head-group hg in 0..1), i.e. tensor
parallelism over heads inside each batch pair (per the to_qkv column / to_out
row sharding hint). Each core projects q/k/v for its 8 heads over the FULL
sequence (so no K/V duplication and no cross-core communication), runs
attention for those heads over all 2048 queries, and computes the row-sharded
half of the output projection. The two partial [2048, 1024] outputs of a pair
are summed on the host during unshard (bias is added on the hg=0 core; the
hg=1 core receives a zero bias so one SPMD program serves all cores).

This head split halves the K and V projection matmul work per core vs the
seq-split variant (which must build full-sequence K/V on both pair cores):
PE streaming work drops from ~918K to ~786K cycles/core.

Layout: all matmul operands fp16 (PSUM f32). V is projected directly in
keys-major layout (stationary = x^T blocks, moving = Wv) so no PE transposes
are needed. Each per-head V block carries 64 ones columns, so the AV matmul
emits the softmax row-sums replicated across PSUM partitions 64..127 for
free; the normalization is then one reciprocal_approx_fast + one multiply on
DVE. Projection work is split into PSUM-tile-sized units and interleaved into
the attention loop (V production streams inside the first attention unit at
one key-tile per step, just ahead of its consumption by the AV chain). The
output projection is split 3+1 over its 4 contraction tiles: the 3-tile half
(heads 0-5, ready after group 2) threads through the last attention group;
only the 1-tile tail trails the attention, overlapped with the output DMA.
"""

import sys
import numpy as np

sys.path.insert(0, "/opt/trn_rl_repo")

B, N, DIM = 4, 2048, 1024
HEADS, DH = 16, 64
NH = 8                # heads per core
SCALE = DH ** -0.5    # 0.125
NC = 8
HALF = N // 2

_compiled = None


def _build():
    import concourse.tile as tile
    from concourse import bacc, mybir

    f32 = mybir.dt.float32
    f16 = mybir.dt.float16
    EXP = mybir.ActivationFunctionType.Exp

    nc = bacc.Bacc("TRN2", target_bir_lowering=False, debug=False, num_devices=NC)

    CT = DIM // 128       # 8 contraction tiles over input channels
    MT = NH * DH // 128   # 4 dim tiles (head-pairs) for kT/qT/ctx
    JT = N // 128         # 16 key tiles
    VW = 128              # per-head v block: 64 dims + 64 ones columns

    # w_q/w_k arrive host-prepacked as [p, m, ct, d] so each m-tile is one
    # contiguous-line DMA (the natural [D, 512] layout would need 256B
    # strided elements — 4x the descriptors and ~4x the issue time).
    X = nc.dram_tensor("x", (DIM, N), f16, kind="ExternalInput").ap()
    WQ = nc.dram_tensor("w_q", (128, MT * CT * 128), f16,
                        kind="ExternalInput").ap()
    WK = nc.dram_tensor("w_k", (128, MT * CT * 128), f16,
                        kind="ExternalInput").ap()
    WV = nc.dram_tensor("w_v", (DIM, NH * DH), f16, kind="ExternalInput").ap()
    WO = nc.dram_tensor("w_out", (NH * DH, DIM), f16, kind="ExternalInput").ap()
    BOUT = nc.dram_tensor("b_out", (DIM,), f32, kind="ExternalInput").ap()
    Y = nc.dram_tensor("y", (N, DIM), f16, kind="ExternalOutput").ap()

    with tile.TileContext(nc) as tc:
        with tc.tile_pool(name="persist", bufs=1) as persist, \
             tc.tile_pool(name="attnbuf", bufs=1) as attnbuf, \
             tc.tile_pool(name="wpool", bufs=1) as wpool:

            kT = [persist.tile([128, N], f16, tag="kT", bufs=MT, name=f"kT{m}")
                  for m in range(MT)]
            qT = [persist.tile([128, N], f16, tag="qT", bufs=MT,
                               name=f"qT{m}") for m in range(MT)]
            v_ext = [persist.tile([128, NH * VW], f16, tag="vext", bufs=JT,
                                  name=f"vext{t}") for t in range(JT)]
            ctx = [persist.tile([128, N], f16, tag="ctx", bufs=MT,
                                name=f"ctx{m}") for m in range(MT)]

            # bias broadcast to all partitions once (zeros on hg=1 cores)
            bias_src = persist.tile([1, DIM], f32, tag="bias_src")
            nc.sync.dma_start(bias_src[:], BOUT.rearrange("(o d) -> o d", o=1))
            bias = persist.tile([128, DIM], f32, tag="bias")
            nc.gpsimd.partition_broadcast(bias[:], bias_src[0:1, :])

            # prefire the exp table load off the critical path
            dummy = attnbuf.tile([1, 8], f16, tag="dummy")
            nc.scalar.activation(dummy[:], bias_src[0:1, 0:8], EXP,
                                 bias=0.0, scale=1.0)

            # ones columns of v_ext (disjoint from the V-projection writes)
            for t in range(JT):
                ones_col = v_ext[t].rearrange(
                    "p (hh c) -> p hh c", c=VW)[:, :, DH:VW]
                nc.gpsimd.memset(ones_col, 1.0)

            with tc.tile_pool(name="psB", bufs=1, space="PSUM") as psB, \
                 tc.tile_pool(name="psInt", bufs=1, space="PSUM") as psInt:
                stage_cm = tc.tile_pool(name="stage", bufs=1)
                stage = stage_cm.__enter__()

                def w_col(src, m):
                    """[128, 8, 128] view of prepacked src for m-tile m."""
                    return src[:, m * CT * 128:(m + 1) * CT * 128].rearrange(
                        "p (t d) -> p t d", d=128)

                # ---- weight + x staging, ordered for fastest first matmul.
                # Few large contiguous-line DMAs: instruction issue (~0.7us
                # per DMA_DIRECT2D) dominates the startup, not bandwidth.
                # x tiles alternate between both HWDGE queues (SP +
                # Activation) — the scalar engine is idle until the first exp.
                wt_k = [wpool.tile([128, CT, 128], f16, tag="wkq",
                                   bufs=2 * MT, name=f"wk{m}") for m in range(MT)]
                wt_q = [wpool.tile([128, CT, 128], f16, tag="wkq",
                                   bufs=2 * MT, name=f"wq{m}") for m in range(MT)]
                xbT = [stage.tile([128, N], f16, tag="xbT", bufs=CT,
                                  name=f"xbT{ct}") for ct in range(CT)]
                wv_all = stage.tile([128, CT, 512], f16, tag="wv")

                nc.sync.dma_start(wt_k[0][:], w_col(WK, 0))
                for ct in range(CT):
                    eng = nc.scalar if ct % 2 else nc.sync
                    eng.dma_start(xbT[ct][:], X[ct * 128:(ct + 1) * 128, :])
                nc.sync.dma_start(wt_q[0][:], w_col(WQ, 0))
                nc.scalar.dma_start(
                    wv_all[:], WV.rearrange("(c p) d -> p c d", p=128))
                nc.sync.dma_start(wt_k[1][:], w_col(WK, 1))
                nc.sync.dma_start(wt_q[1][:], w_col(WQ, 1))
                for m in (2, 3):
                    nc.sync.dma_start(wt_k[m][:], w_col(WK, m))
                    nc.sync.dma_start(wt_q[m][:], w_col(WQ, m))

                # ---- projection units: one PSUM-tile lifecycle each ----
                def kq_unit(wt, dst, s):
                    """dst[:, s*512 : +512] = W[:, m-block].T @ x^T."""
                    ps = psInt.tile([128, 512], f32, tag="pint", bufs=2,
                                    name=f"pi_{dst.name}_{s}")
                    for ct in range(CT):
                        nc.tensor.matmul(ps[:],
                                         wt[:, ct, :],
                                         xbT[ct][:, s * 512:(s + 1) * 512],
                                         start=(ct == 0), stop=(ct == CT - 1))
                    nc.vector.tensor_copy(
                        dst[:, s * 512:(s + 1) * 512], ps[:])

                def v_unit(t):
                    """v_ext[t] (all 8 heads) from x^T key block t."""
                    ps = psInt.tile([128, 512], f32, tag="pint", bufs=2,
                                    name=f"pv{t}")
                    for ct in range(CT):
                        nc.tensor.matmul(
                            ps[:],
                            xbT[ct][:, t * 128:(t + 1) * 128],
                            wv_all[:, ct, :],
                            start=(ct == 0), stop=(ct == CT - 1))
                    dst = v_ext[t].rearrange("p (hh c) -> p hh c", c=VW)[
                        :, :, 0:DH]
                    nc.vector.tensor_copy(dst, ps.rearrange(
                        "p (hh c) -> p hh c", c=DH))

                pending = []

                # ---- attention for one (head, query-half) unit ----
                def attn_unit(h, sq, interleave, u_idx, pace=4,
                              inline_v=None, last=False):
                    hp, p = divmod(h, 2)
                    po = psB.tile([128, 1024], f32, tag="po", bufs=1,
                                  name=f"po{h}_{sq}")
                    ats = {}

                    def av(j):
                        at = ats.pop(j)
                        for u in (0, 1):
                            nc.tensor.matmul(
                                po[:, u * 512:(u + 1) * 512],
                                v_ext[j][:, h * VW:(h + 1) * VW],
                                at[:, u * 512:(u + 1) * 512],
                                start=(j == 0), stop=(j == JT - 1))

                    # AV runs in lagged chains of 4 (av(jt-5..jt-1) at
                    # jt%4==1, jt>=5): all but the first av of a run continue
                    # the po accumulation chain back-to-back, so their
                    # LDWEIGHTS pipeline like mid-chain loads instead of
                    # paying the ~95ns group-entry stall.  The lag keeps the
                    # exp producer well ahead.
                    for jt in range(JT):
                        pp = psB.tile([128, 1024], f32, tag="pp", bufs=2,
                                      name=f"pp{h}_{sq}_{jt}")
                        for u in (0, 1):
                            nc.tensor.matmul(
                                pp[:, u * 512:(u + 1) * 512],
                                kT[hp][p * 64:(p + 1) * 64,
                                       jt * 128:(jt + 1) * 128],
                                qT[hp][p * 64:(p + 1) * 64,
                                       sq * 1024 + u * 512:
                                       sq * 1024 + (u + 1) * 512],
                                start=True, stop=True)
                        at = attnbuf.tile([128, 1024], f16, tag="at", bufs=6,
                                          name=f"at{h}_{sq}_{jt}")
                        nc.scalar.activation(at[:], pp[:], EXP,
                                             bias=0.0, scale=SCALE)
                        ats[jt] = at
                        if jt % 4 == 1 and jt >= 5:
                            for j in range(jt - 5, jt - 1):
                                av(j)
                        if inline_v:
                            for u in inline_v.get(jt, ()):
                                u()
                        if interleave:
                            want = (u_idx * JT + jt + 1) * interleave[0] \
                                // (pace * JT)
                            while interleave[0] - len(pending) < want and pending:
                                pending.pop(0)()
                    for j in range(JT - 4, JT):
                        av(j)
                    # evacuate po with two quick copies so the next unit's AV
                    # chain gets the PSUM bank back ASAP; the reciprocal and
                    # the normalizing multiply run off the critical path.
                    # (reciprocal_approx_fast mis-reads partition-offset PSUM
                    # APs, and SBUF*SBUF tensor ops need equal input base
                    # partitions, so both operands stage at partitions 0..63.)
                    dstc = ctx[hp][p * 64:(p + 1) * 64,
                                   sq * 1024:(sq + 1) * 1024]
                    ss = attnbuf.tile([64, 1024], f32, tag="ss", bufs=1,
                                      name=f"ss{h}_{sq}")
                    rb = attnbuf.tile([64, 1024], f32, tag="rb", bufs=1,
                                      name=f"rb{h}_{sq}")
                    if last:
                        # split by column halves so the output-projection tail
                        # (which consumes ctx columns in ascending order) can
                        # start ~2us earlier.
                        for c in (0, 1):
                            cs = slice(c * 512, (c + 1) * 512)
                            nc.vector.tensor_copy(ss[:, cs], po[64:128, cs])
                            nc.vector.reciprocal_approx_fast(rb[:, cs], ss[:, cs])
                            nc.vector.tensor_mul(dstc[:, cs], po[0:64, cs],
                                                 rb[:, cs])
                        return
                    cu = attnbuf.tile([64, 1024], f16, tag="cu", bufs=1,
                                      name=f"cu{h}_{sq}")
                    nc.vector.tensor_copy(cu[:], po[0:64, :])
                    nc.vector.tensor_copy(ss[:], po[64:128, :])
                    nc.vector.reciprocal_approx_fast(rb[:], ss[:])
                    nc.vector.tensor_mul(dstc, cu[:], rb[:])

                def kq_units_for(m):
                    return ([lambda s=s, m=m: kq_unit(wt_k[m], kT[m], s)
                             for s in range(4)]
                            + [lambda s=s, m=m: kq_unit(wt_q[m], qT[m], s)
                               for s in range(4)])

                # P0: bare minimum for attention unit (h=0, sq=0) jt 0..3;
                # kT[0]'s remaining column slices and all later V key tiles
                # stream inside the unit itself, a few steps ahead of use
                for s in (0,):
                    kq_unit(wt_k[0], kT[0], s)
                kq_unit(wt_q[0], qT[0], 0)
                kq_unit(wt_q[0], qT[0], 1)
                v_unit(0)
                inline0 = {jt: [] for jt in range(JT - 1)}
                for jt in range(JT - 1):
                    inline0[jt].append(lambda t=jt + 1: v_unit(t))
                for s in (1, 2, 3):
                    inline0[s - 1].append(
                        lambda s=s: kq_unit(wt_k[0], kT[0], s))

                # groups 0..2; group g runs heads (2g, 2g+1), both query
                # halves; P(g+1) threads through A(g)'s slots.  unit order
                # (2g,0),(2g+1,0),(2g,1),(2g+1,1) so qT[g] s2/s3 (drained
                # during the first two units) are ready for the sq=1 pair.
                for g in range(3):
                    if g == 0:
                        pending.extend(
                            [lambda: kq_unit(wt_q[0], qT[0], 2),
                             lambda: kq_unit(wt_q[0], qT[0], 3)]
                            + kq_units_for(1))
                    else:
                        pending.extend(kq_units_for(g + 1))
                    units = [(2 * g, 0), (2 * g + 1, 0),
                             (2 * g, 1), (2 * g + 1, 1)]
                    for i, (h, sq) in enumerate(units):
                        if g == 0 and i == 0:
                            attn_unit(h, sq, None, 0, inline_v=inline0)
                        elif g == 0:
                            attn_unit(h, sq, [len(pending)] if pending else None,
                                      i - 1, pace=3)
                        else:
                            attn_unit(h, sq, [len(pending)] if pending else None,
                                      i, pace=4)
                    while pending:
                        pending.pop(0)()

                stage_cm.__exit__(None, None, None)
                # stage (x^T, wv) is gone.  Output projection schedule:
                #  - rows 0..1023 (sq=0 ctx, complete after unit (7,0)):
                #    full 4-ct units + output DMA thread through attention
                #    units (6,1) and (7,1), so half the output DMA streams
                #    during the last attention stretch.
                #  - rows 1024..2047: the 3-ct part (heads 0..5, ready at
                #    group-3 entry) threads through units (6,0)/(7,0) into
                #    yps; only the 1-ct ctx[3] tail trails the attention,
                #    overlapped with the remaining output DMA.
                with tc.tile_pool(name="outw", bufs=1) as outw:
                    wo = [outw.tile([128, DIM], f16, tag="wo", bufs=MT,
                                    name=f"wo{ft}") for ft in range(MT)]
                    yps = [outw.tile([128, DIM], f16, tag="yps", bufs=JT // 2,
                                     name=f"yps{qt}") for qt in range(8, JT)]
                    for ft in range(MT):
                        nc.sync.dma_start(
                            wo[ft][:], WO[ft * 128:(ft + 1) * 128, :])

                    def yp_unit(qt, s, f0, f1):
                        """y rows qt, cols s*512 += sum(ft in f0..f1) ctx.T @ wo."""
                        ps = psInt.tile([128, 512], f32, tag="pint", bufs=2,
                                        name=f"py{qt}_{s}_{f0}")
                        for ft in range(f0, f1):
                            nc.tensor.matmul(ps[:],
                                             ctx[ft][:, qt * 128:(qt + 1) * 128],
                                             wo[ft][:, s * 512:(s + 1) * 512],
                                             start=(ft == f0), stop=(ft == f1 - 1))
                        sl = slice(s * 512, (s + 1) * 512)
                        if f0 == 0 and f1 < MT:
                            nc.vector.tensor_add(yps[qt - 8][:, sl], ps[:],
                                                 bias[:, sl])
                            return
                        ysb = outw.tile([128, 512], f16, tag="ysb", bufs=4,
                                        name=f"ysb{qt}_{s}")
                        if f0 == 0:
                            nc.vector.tensor_add(ysb[:], ps[:], bias[:, sl])
                        else:
                            nc.vector.tensor_add(ysb[:], ps[:], yps[qt - 8][:, sl])
                        nc.sync.dma_start(
                            Y[qt * 128:(qt + 1) * 128, sl], ysb[:])

                    # rows 1024..2047: 3-ct partial (heads 0..5) + bias into
                    # yps during units (6,0)/(7,0)
                    pending.extend(lambda qt=qt, s=s: yp_unit(qt, s, 0, 3)
                                   for qt in range(8, JT) for s in (0, 1))
                    ileave = [len(pending)]
                    attn_unit(6, 0, ileave, 0, pace=2)
                    attn_unit(7, 0, ileave, 1, pace=2)
                    while pending:
                        pending.pop(0)()
                    # rows 0..1023: full 4-ct units, DMA'd during (6,1)/(7,1)
                    pending.extend(lambda qt=qt, s=s: yp_unit(qt, s, 0, MT)
                                   for qt in range(8) for s in (0, 1))
                    ileave = [len(pending)]
                    attn_unit(6, 1, ileave, 0, pace=2)
                    attn_unit(7, 1, ileave, 1, pace=2, last=True)
                    while pending:
                        pending.pop(0)()
                    # tail: ctx[3] (heads 6,7) contraction for rows 1024..2047
                    for qt in range(8, JT):
                        for s in (0, 1):
                            yp_unit(qt, s, 3, 4)

    nc.compile()
    return nc


def _get_compiled():
    global _compiled
    if _compiled is None:
        _compiled = _build()
    return _compiled


def _build_in_maps(x, w_qkv, w_out, b_out):
    x = np.asarray(x, dtype=np.float32)
    w_qkv = np.asarray(w_qkv, dtype=np.float16)
    w_out = np.asarray(w_out, dtype=np.float16)
    b_out = np.asarray(b_out, dtype=np.float32)
    zeros = np.zeros_like(b_out)

    def prepack(w):
        # [D, 512] -> [p, m, ct, d] so each m-tile DMA reads contiguous
        # 2KB-per-partition lines on device
        mt, ct = NH * DH // 128, DIM // 128
        return np.ascontiguousarray(
            w.reshape(ct, 128, mt, 128).transpose(1, 2, 0, 3).reshape(
                128, mt * ct * 128))

    xbs = [np.ascontiguousarray(x[b].T.astype(np.float16)) for b in range(B)]
    in_maps = []
    for c in range(NC):
        b, hg = divmod(c, 2)
        cols = slice(hg * NH * DH, (hg + 1) * NH * DH)
        in_maps.append({
            "x": xbs[b],
            "w_q": prepack(w_qkv[:, 0 * DIM:1 * DIM][:, cols]),
            "w_k": prepack(w_qkv[:, 1 * DIM:2 * DIM][:, cols]),
            "w_v": np.ascontiguousarray(w_qkv[:, 2 * DIM:3 * DIM][:, cols]),
            "w_out": np.ascontiguousarray(w_out[cols, :]),
            "b_out": b_out if hg == 0 else zeros,
        })
    return in_maps


def kernel(x, w_qkv, w_out, b_out):
    from concourse.bass_utils import run_bass_kernel_spmd

    nc = _get_compiled()
    in_maps = _build_in_maps(x, w_qkv, w_out, b_out)
    res = run_bass_kernel_spmd(nc, in_maps, core_ids=list(range(NC)))

    out = np.empty((B, N, DIM), dtype=np.float32)
    for b in range(B):
        out[b] = (res.results[2 * b]["y"].astype(np.float32)
                  + res.results[2 * b + 1]["y"].astype(np.float32))
    return out
